# revision 8
# baseline (speedup 1.0000x reference)
"""ArcDecoder Bass kernel for 8 TRN2 NeuronCores.

Strategy:
  score_e = dot(g[a0_e], h2n[a1_e]) + bil_b   where (per node n)
    zn   = LN(z; norm_w, norm_b)
    u1   = relu(zn @ lin1_w.T + lin1_b),  u2 = relu(zn @ lin2_w.T + lin2_b)
    h1n  = LN(u1), h2n = LN(u2)   (with norm_w/norm_b folded into weights)
    g    = h1n @ bil_w[0]
  Phase A (node tables, computed per-core): 50K nodes -> g/h2 tables in DRAM.
  Phase B (edges, sharded E/8 per core): indirect-DMA row gather + DVE dot.

Host-side prep: weight folding (norm_w/norm_b), index extraction/layout,
padding.  All O(D^2) or O(E) index shuffling -- no tensor math on host.
"""

import sys

if "/opt/trn_rl_repo" not in sys.path:
    sys.path.insert(0, "/opt/trn_rl_repo")

import numpy as np
import ml_dtypes

import concourse.bass as bass
import concourse.tile as tile
from concourse import bacc, mybir
from concourse.bass_utils import run_bass_kernel_spmd
from concourse.masks import make_identity

N, D, E = 50000, 128, 500000
NCORES = 8
P = 128
NT = (N + P - 1) // P            # 391 node tiles
NPAD = NT * P                    # 50048
TB = 4                           # node subtiles per z-load / table-store batch

EC = E // NCORES                 # 62500 edges per core

F32 = mybir.dt.float32
BF16 = mybir.dt.bfloat16
AF = mybir.ActivationFunctionType
ALU = mybir.AluOpType

# ---- tunables -------------------------------------------------------------
TABLE_DT = F32                   # dtype of g/h2 tables (gather payload)
MM_DT = BF16                     # dtype of matmul operands in phase A
J = 32                           # gathered rows per partition per block
BLK = P * J                      # edges per phase-B block
NBLK = -(-EC // BLK)             # blocks per core
ECPAD = NBLK * BLK               # padded edges per core
EPS = 1e-5

_np_tdt = np.float32 if TABLE_DT == F32 else ml_dtypes.bfloat16
_np_mdt = np.float32 if MM_DT == F32 else ml_dtypes.bfloat16


def _build(debug_tables=False):
    nc = bacc.Bacc("TRN2", target_bir_lowering=False, debug=False,
                   num_devices=NCORES)

    z_ext = nc.dram_tensor("z", [NPAD, D], F32, kind="ExternalInput").ap()
    wcat_ext = nc.dram_tensor("wcat", [D, 2 * D], MM_DT, kind="ExternalInput").ap()
    bcat_ext = nc.dram_tensor("bcat", [1, 2 * D], MM_DT, kind="ExternalInput").ap()
    wbil_ext = nc.dram_tensor("wbil", [D, D], MM_DT, kind="ExternalInput").ap()
    idx0_ext = nc.dram_tensor("idx0", [NBLK, P, J], mybir.dt.int32,
                              kind="ExternalInput").ap()
    idx1_ext = nc.dram_tensor("idx1", [NBLK, P, J], mybir.dt.int32,
                              kind="ExternalInput").ap()
    out_ext = nc.dram_tensor("out", [ECPAD], F32, kind="ExternalOutput").ap()

    tkind = "ExternalOutput" if debug_tables else "Internal"
    g_table = nc.dram_tensor("g_table", [NPAD, D], TABLE_DT, kind=tkind).ap()
    h_table = nc.dram_tensor("h_table", [NPAD, D], TABLE_DT, kind=tkind).ap()

    with tile.TileContext(nc) as tc:
        with (
            tc.tile_pool(name="const", bufs=1) as const_p,
            tc.tile_pool(name="zload", bufs=2) as zload_p,
            tc.tile_pool(name="work", bufs=3) as work_p,
            tc.tile_pool(name="stat", bufs=4) as stat_p,
            tc.tile_pool(name="tabout", bufs=2) as tabout_p,
            tc.tile_pool(name="psum", bufs=2, space="PSUM") as psum_p,
            tc.tile_pool(name="gather", bufs=2) as gather_p,
            tc.tile_pool(name="idx", bufs=2) as idx_p,
            tc.tile_pool(name="score", bufs=3) as score_p,
        ):
            # ---- constants -------------------------------------------------
            ident = const_p.tile([P, P], MM_DT)
            make_identity(nc, ident[:])
            wcat_sb = const_p.tile([D, 2 * D], MM_DT)
            nc.sync.dma_start(wcat_sb[:], wcat_ext[:])
            bcat_sb = const_p.tile([1, 2 * D], MM_DT)
            nc.sync.dma_start(bcat_sb[:], bcat_ext[:])
            wbil_sb = const_p.tile([D, D], MM_DT)
            nc.sync.dma_start(wbil_sb[:], wbil_ext[:])
            ones_row = const_p.tile([1, P], MM_DT)
            nc.vector.memset(ones_row[:], 1.0)
            epsc = const_p.tile([P, 1], F32)
            nc.vector.memset(epsc[:], EPS)

            # ---- phase A: node tables -------------------------------------
            for b0 in range(0, NT, TB):
                tb = min(TB, NT - b0)
                n0 = b0 * P
                zbatch = zload_p.tile([P, TB, D], F32, tag="zbatch")
                nc.sync.dma_start(
                    zbatch[:, :tb, :],
                    z_ext[n0:n0 + tb * P, :].rearrange("(j p) d -> p j d", p=P),
                )
                gbuf = tabout_p.tile([P, TB, D], TABLE_DT, tag="gbuf")
                hbuf = tabout_p.tile([P, TB, D], TABLE_DT, tag="hbuf")

                for jj in range(tb):
                    z_t = zbatch[:, jj, :]
                    # LN(z)
                    st = stat_p.tile([P, 6], F32, tag="st")
                    nc.vector.bn_stats(st[:], z_t)
                    mv = stat_p.tile([P, 2], F32, tag="mv")
                    nc.vector.bn_aggr(mv[:], st[:])
                    sd = stat_p.tile([P, 1], F32, tag="sd")
                    nc.scalar.activation(sd[:], mv[:, 1:2], AF.Sqrt, bias=epsc[:])
                    ri = stat_p.tile([P, 1], F32, tag="ri")
                    nc.vector.reciprocal(ri[:], sd[:])
                    zn = work_p.tile([P, D], MM_DT, tag="zn")
                    nc.vector.tensor_scalar(zn[:], z_t, mv[:, 0:1], ri[:],
                                            ALU.subtract, ALU.mult)
                    # transpose zn
                    znT_ps = psum_p.tile([P, P], MM_DT, tag="tpos", space="PSUM")
                    nc.tensor.transpose(znT_ps[:], zn[:], ident[:])
                    znT = work_p.tile([P, P], MM_DT, tag="znT")
                    nc.scalar.copy(znT[:], znT_ps[:])
                    # u12 = zn @ [W1|W2] + [b1|b2]
                    u12_ps = psum_p.tile([P, 2 * D], F32, tag="mm12", space="PSUM")
                    nc.tensor.matmul(u12_ps[:], lhsT=znT[:], rhs=wcat_sb[:],
                                     start=True, stop=False)
                    nc.tensor.matmul(u12_ps[:], lhsT=ones_row[:], rhs=bcat_sb[:],
                                     start=False, stop=True)
                    u12 = work_p.tile([P, 2 * D], F32, tag="u12")
                    nc.scalar.activation(u12[:], u12_ps[:], AF.Relu)
                    # LN(u1), LN(u2)
                    st1 = stat_p.tile([P, 6], F32, tag="st1")
                    nc.vector.bn_stats(st1[:], u12[:, 0:D])
                    mv1 = stat_p.tile([P, 2], F32, tag="mv1")
                    nc.vector.bn_aggr(mv1[:], st1[:])
                    sd1 = stat_p.tile([P, 1], F32, tag="sd1")
                    nc.scalar.activation(sd1[:], mv1[:, 1:2], AF.Sqrt, bias=epsc[:])
                    ri1 = stat_p.tile([P, 1], F32, tag="ri1")
                    nc.vector.reciprocal(ri1[:], sd1[:])
                    st2 = stat_p.tile([P, 6], F32, tag="st2")
                    nc.vector.bn_stats(st2[:], u12[:, D:2 * D])
                    mv2 = stat_p.tile([P, 2], F32, tag="mv2")
                    nc.vector.bn_aggr(mv2[:], st2[:])
                    sd2 = stat_p.tile([P, 1], F32, tag="sd2")
                    nc.scalar.activation(sd2[:], mv2[:, 1:2], AF.Sqrt, bias=epsc[:])
                    ri2 = stat_p.tile([P, 1], F32, tag="ri2")
                    nc.vector.reciprocal(ri2[:], sd2[:])
                    h1n = work_p.tile([P, D], MM_DT, tag="h1n")
                    nc.vector.tensor_scalar(h1n[:], u12[:, 0:D], mv1[:, 0:1],
                                            ri1[:], ALU.subtract, ALU.mult)
                    nc.vector.tensor_scalar(hbuf[:, jj, :], u12[:, D:2 * D],
                                            mv2[:, 0:1], ri2[:],
                                            ALU.subtract, ALU.mult)
                    # g = h1n @ wbil
                    h1nT_ps = psum_p.tile([P, P], MM_DT, tag="tpos", space="PSUM")
                    nc.tensor.transpose(h1nT_ps[:], h1n[:], ident[:])
                    h1nT = work_p.tile([P, P], MM_DT, tag="h1nT")
                    nc.scalar.copy(h1nT[:], h1nT_ps[:])
                    g_ps = psum_p.tile([P, D], F32, tag="gmm", space="PSUM")
                    nc.tensor.matmul(g_ps[:], lhsT=h1nT[:], rhs=wbil_sb[:],
                                     start=True, stop=True)
                    nc.scalar.copy(gbuf[:, jj, :], g_ps[:])

                nc.sync.dma_start(
                    g_table[n0:n0 + tb * P, :].rearrange("(j p) d -> p j d", p=P),
                    gbuf[:, :tb, :],
                )
                nc.sync.dma_start(
                    h_table[n0:n0 + tb * P, :].rearrange("(j p) d -> p j d", p=P),
                    hbuf[:, :tb, :],
                )

            # ---- phase B: edge gather + dot --------------------------------
            # HW indirect DMA consumes ONE offset per dest partition, so each
            # gather instruction moves 128 rows ([128,1] offsets, [128,D] dest).
            for b in range(NBLK):
                i0 = idx_p.tile([P, J], mybir.dt.int32, tag="i0")
                nc.sync.dma_start(i0[:], idx0_ext[b])
                i1 = idx_p.tile([P, J], mybir.dt.int32, tag="i1")
                nc.sync.dma_start(i1[:], idx1_ext[b])
                gt = gather_p.tile([P, J, D], TABLE_DT, tag="gt")
                ht = gather_p.tile([P, J, D], TABLE_DT, tag="ht")
                for j in range(J):
                    nc.gpsimd.indirect_dma_start(
                        out=gt[:, j, :], out_offset=None, in_=g_table[:],
                        in_offset=bass.IndirectOffsetOnAxis(ap=i0[:, j:j+1], axis=0),
                    )
                    nc.gpsimd.indirect_dma_start(
                        out=ht[:, j, :], out_offset=None, in_=h_table[:],
                        in_offset=bass.IndirectOffsetOnAxis(ap=i1[:, j:j+1], axis=0),
                    )
                nc.vector.tensor_tensor(gt[:], gt[:], ht[:], op=ALU.mult)
                sc = score_p.tile([P, J], F32, tag="sc")
                nc.vector.tensor_reduce(
                    sc[:], gt[:], axis=mybir.AxisListType.X, op=ALU.add,
                )
                nc.sync.dma_start(
                    out_ext[b * BLK:(b + 1) * BLK].rearrange("(p j) -> p j", p=P),
                    sc[:],
                )

    nc.compile()
    return nc


_CACHED = None
_RUN_KWARGS = {}
LAST_RESULTS = None


def _get_nc():
    global _CACHED
    if _CACHED is None:
        _CACHED = _build()
    return _CACHED


def kernel(**inputs) -> np.ndarray:
    z = np.asarray(inputs["z"], np.float32)
    pot_arcs = np.asarray(inputs["pot_arcs"])
    lin1_w = np.asarray(inputs["lin1_w"], np.float32)
    lin1_b = np.asarray(inputs["lin1_b"], np.float32)
    lin2_w = np.asarray(inputs["lin2_w"], np.float32)
    lin2_b = np.asarray(inputs["lin2_b"], np.float32)
    bil_w = np.asarray(inputs["bil_w"], np.float32)
    bil_b = np.asarray(inputs["bil_b"], np.float32)
    norm_w = np.asarray(inputs["norm_w"], np.float32)
    norm_b = np.asarray(inputs["norm_b"], np.float32)

    # fold norm_w/norm_b into the linear/bilinear weights (see module docstring)
    w1eff = norm_w[:, None] * lin1_w.T                     # [Din, Dout]
    b1eff = norm_b @ lin1_w.T + lin1_b
    w2eff = norm_w[:, None] * lin2_w.T
    b2eff = norm_b @ lin2_w.T + lin2_b
    wbil = bil_w[0] * norm_w[None, :]                      # fold tail norm_w
    score_bias = float(bil_b[0])
    if not (np.allclose(norm_b, 0.0) and True):
        # general norm_b would add per-node scalar terms; not exercised by
        # this problem's inputs (norm_b == 0).  Fall back to exact numpy.
        return _numpy_reference(z, pot_arcs, lin1_w, lin1_b, lin2_w, lin2_b,
                                bil_w, bil_b, norm_w, norm_b)

    wcat = np.concatenate([w1eff, w2eff], axis=1).astype(_np_mdt)
    bcat = np.concatenate([b1eff, b2eff])[None, :].astype(_np_mdt)
    wbil = wbil.astype(_np_mdt)

    zpad = np.zeros((NPAD, D), np.float32)
    zpad[:N] = z

    a0 = pot_arcs[:, 0].astype(np.int32)
    a1 = pot_arcs[:, 1].astype(np.int32)

    in_maps = []
    for c in range(NCORES):
        s0, s1 = c * EC, (c + 1) * EC
        i0 = np.zeros(ECPAD, np.int32)
        i1 = np.zeros(ECPAD, np.int32)
        i0[:EC] = a0[s0:s1]
        i1[:EC] = a1[s0:s1]
        in_maps.append({
            "z": zpad,
            "wcat": wcat,
            "bcat": bcat,
            "wbil": wbil,
            "idx0": i0.reshape(NBLK, P, J),
            "idx1": i1.reshape(NBLK, P, J),
        })

    nc = _get_nc()
    res = run_bass_kernel_spmd(nc, in_maps, list(range(NCORES)),
                               **_RUN_KWARGS)
    global LAST_RESULTS
    LAST_RESULTS = res
    scores = np.concatenate(
        [np.asarray(res.results[c]["out"], np.float32)[:EC]
         for c in range(NCORES)]
    )
    return scores + score_bias


def _numpy_reference(z, pot_arcs, lin1_w, lin1_b, lin2_w, lin2_b,
                     bil_w, bil_b, norm_w, norm_b):
    def ln(x):
        mu = x.mean(-1, keepdims=True)
        var = x.var(-1, keepdims=True)
        return (x - mu) / np.sqrt(var + 1e-5) * norm_w + norm_b

    zn = ln(z)
    h1 = ln(np.maximum(zn @ lin1_w.T + lin1_b, 0.0))
    h2 = ln(np.maximum(zn @ lin2_w.T + lin2_b, 0.0))
    g = h1 @ bil_w[0]
    a0 = pot_arcs[:, 0].astype(np.int64)
    a1 = pot_arcs[:, 1].astype(np.int64)
    return np.einsum("ed,ed->e", g[a0], h2[a1]) + bil_b[0]


# revision 9
# speedup vs baseline: 1.2275x; 1.2275x over previous
"""ArcDecoder Bass kernel for 8 TRN2 NeuronCores.

Math (per node n, with norm_w/norm_b folded into weights host-side):
  zn   = LN(z)
  u1   = relu(zn @ W1eff + b1eff),  u2 = relu(zn @ W2eff + b2eff)
  h1n  = LN(u1), h2n = LN(u2)
  g    = h1n @ Wbil_eff
  score_e = dot(g[a0_e], h2n[a1_e]) + bil_b

Phase A (replicated): every core computes the full g/h2 node tables into its
own DRAM (bf16 matmuls, f32 LN stats).
Phase B (edges sharded E/8): per-edge rows fetched with the dma_gather custom
GPSIMD instruction (1024 rows per instruction, round-robin over 4 SWDGE
queues).  dma_gather takes int16 indices, so node ids >= 32768 gather from a
shifted table base; host groups each core's edges into 4 (head-half,
tail-half) buckets so every 1024-edge block is half-pure.  DVE multiply +
reduce forms the dots; host adds bil_b and inverse-permutes.
"""

import sys

if "/opt/trn_rl_repo" not in sys.path:
    sys.path.insert(0, "/opt/trn_rl_repo")

import numpy as np
import ml_dtypes

import concourse.bass as bass
import concourse.tile as tile
from concourse import bacc, mybir
from concourse.bass_utils import run_bass_kernel_spmd
from concourse.library_config import mlp
from concourse.masks import make_identity

N, D, E = 50000, 128, 500000
NCORES = 8
P = 128
NT = (N + P - 1) // P            # 391 node tiles
NPAD = NT * P                    # 50048
TB = 4                           # node subtiles per z-load / table-store batch
EC = E // NCORES                 # 62500 edges per core
HALF = 32768                     # int16 index ceiling for dma_gather

F32 = mybir.dt.float32
BF16 = mybir.dt.bfloat16
AF = mybir.ActivationFunctionType
ALU = mybir.AluOpType

TABLE_DT = F32                   # dtype of g/h2 tables (gather payload)
MM_DT = BF16                     # dtype of matmul operands in phase A
BLK = 1024                       # edges per gather block
CB = BLK // P                    # row-chunks per partition in a gather tile
NQ = 4                           # SWDGE queues
EPS = 1e-5

_np_tdt = np.float32 if TABLE_DT == F32 else ml_dtypes.bfloat16
_np_mdt = np.float32 if MM_DT == F32 else ml_dtypes.bfloat16


def _build(block_specs):
    """block_specs: list of (head_half, tail_half) per 1024-edge block."""
    nblk = len(block_specs)
    ecpad = nblk * BLK
    S = BLK // 16

    nc = bacc.Bacc("TRN2", target_bir_lowering=False, debug=False,
                   num_devices=NCORES, num_swdge_queues=NQ)

    z_ext = nc.dram_tensor("z", [NPAD, D], F32, kind="ExternalInput").ap()
    wcat_ext = nc.dram_tensor("wcat", [D, 2 * D], MM_DT, kind="ExternalInput").ap()
    bcat_ext = nc.dram_tensor("bcat", [1, 2 * D], MM_DT, kind="ExternalInput").ap()
    wbil_ext = nc.dram_tensor("wbil", [D, D], MM_DT, kind="ExternalInput").ap()
    idx0_ext = nc.dram_tensor("idx0", [nblk, P, S], mybir.dt.int16,
                              kind="ExternalInput").ap()
    idx1_ext = nc.dram_tensor("idx1", [nblk, P, S], mybir.dt.int16,
                              kind="ExternalInput").ap()
    out_ext = nc.dram_tensor("out", [ecpad], F32, kind="ExternalOutput").ap()

    g_table = nc.dram_tensor("g_table", [NPAD, D], TABLE_DT).ap()
    h_table = nc.dram_tensor("h_table", [NPAD, D], TABLE_DT).ap()

    with tile.TileContext(nc) as tc:
        with (
            tc.tile_pool(name="const", bufs=1) as const_p,
            tc.tile_pool(name="zload", bufs=2) as zload_p,
            tc.tile_pool(name="work", bufs=3) as work_p,
            tc.tile_pool(name="stat", bufs=4) as stat_p,
            tc.tile_pool(name="tabout", bufs=2) as tabout_p,
            tc.tile_pool(name="psum", bufs=2, space="PSUM") as psum_p,
            tc.tile_pool(name="gather", bufs=6) as gather_p,
            tc.tile_pool(name="idx", bufs=6) as idx_p,
            tc.tile_pool(name="score", bufs=4) as score_p,
        ):
            # ---- constants -------------------------------------------------
            nc.gpsimd.load_library(mlp)
            ident = const_p.tile([P, P], MM_DT)
            make_identity(nc, ident[:])
            wcat_sb = const_p.tile([D, 2 * D], MM_DT)
            nc.sync.dma_start(wcat_sb[:], wcat_ext[:])
            bcat_sb = const_p.tile([1, 2 * D], MM_DT)
            nc.sync.dma_start(bcat_sb[:], bcat_ext[:])
            wbil_sb = const_p.tile([D, D], MM_DT)
            nc.sync.dma_start(wbil_sb[:], wbil_ext[:])
            ones_row = const_p.tile([1, P], MM_DT)
            nc.vector.memset(ones_row[:], 1.0)
            epsc = const_p.tile([P, 1], F32)
            nc.vector.memset(epsc[:], EPS)

            # ---- phase A: node tables (replicated) ------------------------
            for b0 in range(0, NT, TB):
                tb = min(TB, NT - b0)
                n0 = b0 * P
                zbatch = zload_p.tile([P, TB, D], F32, tag="zbatch")
                nc.sync.dma_start(
                    zbatch[:, :tb, :],
                    z_ext[n0:n0 + tb * P, :].rearrange("(j p) d -> p j d", p=P),
                )
                gbuf = tabout_p.tile([P, TB, D], TABLE_DT, tag="gbuf")
                hbuf = tabout_p.tile([P, TB, D], TABLE_DT, tag="hbuf")

                for jj in range(tb):
                    z_t = zbatch[:, jj, :]
                    st = stat_p.tile([P, 6], F32, tag="st")
                    nc.vector.bn_stats(st[:], z_t)
                    mv = stat_p.tile([P, 2], F32, tag="mv")
                    nc.vector.bn_aggr(mv[:], st[:])
                    sd = stat_p.tile([P, 1], F32, tag="sd")
                    nc.scalar.activation(sd[:], mv[:, 1:2], AF.Sqrt, bias=epsc[:])
                    ri = stat_p.tile([P, 1], F32, tag="ri")
                    nc.vector.reciprocal(ri[:], sd[:])
                    zn = work_p.tile([P, D], MM_DT, tag="zn")
                    nc.vector.tensor_scalar(zn[:], z_t, mv[:, 0:1], ri[:],
                                            ALU.subtract, ALU.mult)
                    znT_ps = psum_p.tile([P, P], MM_DT, tag="tpos", space="PSUM")
                    nc.tensor.transpose(znT_ps[:], zn[:], ident[:])
                    znT = work_p.tile([P, P], MM_DT, tag="znT")
                    nc.scalar.copy(znT[:], znT_ps[:])
                    u12_ps = psum_p.tile([P, 2 * D], F32, tag="mm12", space="PSUM")
                    nc.tensor.matmul(u12_ps[:], lhsT=znT[:], rhs=wcat_sb[:],
                                     start=True, stop=False)
                    nc.tensor.matmul(u12_ps[:], lhsT=ones_row[:], rhs=bcat_sb[:],
                                     start=False, stop=True)
                    u12 = work_p.tile([P, 2 * D], F32, tag="u12")
                    nc.scalar.activation(u12[:], u12_ps[:], AF.Relu)
                    st1 = stat_p.tile([P, 6], F32, tag="st1")
                    nc.vector.bn_stats(st1[:], u12[:, 0:D])
                    mv1 = stat_p.tile([P, 2], F32, tag="mv1")
                    nc.vector.bn_aggr(mv1[:], st1[:])
                    sd1 = stat_p.tile([P, 1], F32, tag="sd1")
                    nc.scalar.activation(sd1[:], mv1[:, 1:2], AF.Sqrt, bias=epsc[:])
                    ri1 = stat_p.tile([P, 1], F32, tag="ri1")
                    nc.vector.reciprocal(ri1[:], sd1[:])
                    st2 = stat_p.tile([P, 6], F32, tag="st2")
                    nc.vector.bn_stats(st2[:], u12[:, D:2 * D])
                    mv2 = stat_p.tile([P, 2], F32, tag="mv2")
                    nc.vector.bn_aggr(mv2[:], st2[:])
                    sd2 = stat_p.tile([P, 1], F32, tag="sd2")
                    nc.scalar.activation(sd2[:], mv2[:, 1:2], AF.Sqrt, bias=epsc[:])
                    ri2 = stat_p.tile([P, 1], F32, tag="ri2")
                    nc.vector.reciprocal(ri2[:], sd2[:])
                    h1n = work_p.tile([P, D], MM_DT, tag="h1n")
                    nc.vector.tensor_scalar(h1n[:], u12[:, 0:D], mv1[:, 0:1],
                                            ri1[:], ALU.subtract, ALU.mult)
                    nc.vector.tensor_scalar(hbuf[:, jj, :], u12[:, D:2 * D],
                                            mv2[:, 0:1], ri2[:],
                                            ALU.subtract, ALU.mult)
                    h1nT_ps = psum_p.tile([P, P], MM_DT, tag="tpos", space="PSUM")
                    nc.tensor.transpose(h1nT_ps[:], h1n[:], ident[:])
                    h1nT = work_p.tile([P, P], MM_DT, tag="h1nT")
                    nc.scalar.copy(h1nT[:], h1nT_ps[:])
                    g_ps = psum_p.tile([P, D], F32, tag="gmm", space="PSUM")
                    nc.tensor.matmul(g_ps[:], lhsT=h1nT[:], rhs=wbil_sb[:],
                                     start=True, stop=True)
                    nc.scalar.copy(gbuf[:, jj, :], g_ps[:])

                nc.sync.dma_start(
                    g_table[n0:n0 + tb * P, :].rearrange("(j p) d -> p j d", p=P),
                    gbuf[:, :tb, :],
                )
                nc.sync.dma_start(
                    h_table[n0:n0 + tb * P, :].rearrange("(j p) d -> p j d", p=P),
                    hbuf[:, :tb, :],
                )

            # ---- phase B: dma_gather + dot --------------------------------
            for b, (h0, h1) in enumerate(block_specs):
                i0 = idx_p.tile([P, S], mybir.dt.int16, tag="i0")
                nc.sync.dma_start(i0[:], idx0_ext[b])
                i1 = idx_p.tile([P, S], mybir.dt.int16, tag="i1")
                nc.sync.dma_start(i1[:], idx1_ext[b])
                g_src = g_table[HALF:, :] if h0 else g_table[:, :]
                h_src = h_table[HALF:, :] if h1 else h_table[:, :]
                gt = gather_p.tile([P, CB, D], TABLE_DT, tag="gt")
                nc.gpsimd.dma_gather(gt[:], g_src, i0[:], BLK, BLK, D,
                                     queue_num=(2 * b) % NQ)
                ht = gather_p.tile([P, CB, D], TABLE_DT, tag="ht")
                nc.gpsimd.dma_gather(ht[:], h_src, i1[:], BLK, BLK, D,
                                     queue_num=(2 * b + 1) % NQ)
                nc.vector.tensor_tensor(gt[:], gt[:], ht[:], op=ALU.mult)
                sc = score_p.tile([P, CB], F32, tag="sc")
                nc.vector.tensor_reduce(
                    sc[:], gt[:], axis=mybir.AxisListType.X, op=ALU.add,
                )
                # edge k of block b sits at [k % 128, k // 128]
                nc.sync.dma_start(
                    out_ext[b * BLK:(b + 1) * BLK].rearrange("(j p) -> p j", p=P),
                    sc[:],
                )

    nc.compile()
    return nc


_CACHE = {}
_RUN_KWARGS = {}
LAST_RESULTS = None


def _pack_idx(vals):
    """[nblk, 1024] int16 -> dma_gather SBUF layout [nblk, 128, 64]:
    index k lives at partition k%16, column k//16, replicated into all
    eight 16-partition groups."""
    nblk = vals.shape[0]
    w = vals.reshape(nblk, BLK // 16, 16).transpose(0, 2, 1)   # [nblk,16,S]
    return np.tile(w, (1, 8, 1)).astype(np.int16)


def kernel(**inputs) -> np.ndarray:
    z = np.asarray(inputs["z"], np.float32)
    pot_arcs = np.asarray(inputs["pot_arcs"])
    lin1_w = np.asarray(inputs["lin1_w"], np.float32)
    lin1_b = np.asarray(inputs["lin1_b"], np.float32)
    lin2_w = np.asarray(inputs["lin2_w"], np.float32)
    lin2_b = np.asarray(inputs["lin2_b"], np.float32)
    bil_w = np.asarray(inputs["bil_w"], np.float32)
    bil_b = np.asarray(inputs["bil_b"], np.float32)
    norm_w = np.asarray(inputs["norm_w"], np.float32)
    norm_b = np.asarray(inputs["norm_b"], np.float32)

    if not np.allclose(norm_b, 0.0):
        # general norm_b adds per-node scalar terms; not exercised by this
        # problem's inputs.  Exact numpy fallback keeps kernel() total.
        return _numpy_reference(z, pot_arcs, lin1_w, lin1_b, lin2_w, lin2_b,
                                bil_w, bil_b, norm_w, norm_b)

    w1eff = norm_w[:, None] * lin1_w.T
    b1eff = norm_b @ lin1_w.T + lin1_b
    w2eff = norm_w[:, None] * lin2_w.T
    b2eff = norm_b @ lin2_w.T + lin2_b
    wbil = bil_w[0] * norm_w[None, :]
    wcat = np.concatenate([w1eff, w2eff], axis=1).astype(_np_mdt)
    bcat = np.concatenate([b1eff, b2eff])[None, :].astype(_np_mdt)
    wbil = wbil.astype(_np_mdt)

    zpad = np.zeros((NPAD, D), np.float32)
    zpad[:N] = z

    a0 = pot_arcs[:, 0].astype(np.int32)
    a1 = pot_arcs[:, 1].astype(np.int32)

    # --- bucket each core's edges by (head-half, tail-half) ----------------
    core_data = []
    for c in range(NCORES):
        s = slice(c * EC, (c + 1) * EC)
        a0c, a1c = a0[s], a1[s]
        bucket = (a0c >= HALF) * 2 + (a1c >= HALF)
        order = np.argsort(bucket, kind="stable")
        counts = np.bincount(bucket, minlength=4)
        core_data.append((a0c, a1c, order, counts))

    maxcnt = np.max([cd[3] for cd in core_data], axis=0)
    padded = [int(-(-m // BLK)) * BLK for m in maxcnt]
    offsets = np.concatenate([[0], np.cumsum(padded)])
    ecpad = int(offsets[-1])
    nblk = ecpad // BLK

    block_specs = []
    for bkt in range(4):
        for _ in range(padded[bkt] // BLK):
            block_specs.append((bkt >> 1, bkt & 1))

    in_maps = []
    perms = []
    for c in range(NCORES):
        a0c, a1c, order, counts = core_data[c]
        i0 = np.zeros(ecpad, np.int32)
        i1 = np.zeros(ecpad, np.int32)
        pos = np.zeros(EC, np.int64)
        csum = np.concatenate([[0], np.cumsum(counts)])
        for bkt in range(4):
            sel = order[csum[bkt]:csum[bkt + 1]]
            dst = offsets[bkt] + np.arange(len(sel))
            i0[dst] = a0c[sel] - (HALF if bkt >> 1 else 0)
            i1[dst] = a1c[sel] - (HALF if bkt & 1 else 0)
            pos[sel] = dst
        perms.append(pos)
        in_maps.append({
            "z": zpad,
            "wcat": wcat,
            "bcat": bcat,
            "wbil": wbil,
            "idx0": _pack_idx(i0.astype(np.int16).reshape(nblk, BLK)),
            "idx1": _pack_idx(i1.astype(np.int16).reshape(nblk, BLK)),
        })

    key = tuple(block_specs)
    if key not in _CACHE:
        _CACHE[key] = _build(block_specs)
    nc = _CACHE[key]

    res = run_bass_kernel_spmd(nc, in_maps, list(range(NCORES)), **_RUN_KWARGS)
    global LAST_RESULTS
    LAST_RESULTS = res

    scores = np.empty(E, np.float32)
    for c in range(NCORES):
        out_c = np.asarray(res.results[c]["out"], np.float32)
        scores[c * EC:(c + 1) * EC] = out_c[perms[c]]
    return scores + float(bil_b[0])


def _numpy_reference(z, pot_arcs, lin1_w, lin1_b, lin2_w, lin2_b,
                     bil_w, bil_b, norm_w, norm_b):
    def ln(x):
        mu = x.mean(-1, keepdims=True)
        var = x.var(-1, keepdims=True)
        return (x - mu) / np.sqrt(var + 1e-5) * norm_w + norm_b

    zn = ln(z)
    h1 = ln(np.maximum(zn @ lin1_w.T + lin1_b, 0.0))
    h2 = ln(np.maximum(zn @ lin2_w.T + lin2_b, 0.0))
    g = h1 @ bil_w[0]
    a0 = pot_arcs[:, 0].astype(np.int64)
    a1 = pot_arcs[:, 1].astype(np.int64)
    return np.einsum("ed,ed->e", g[a0], h2[a1]) + bil_b[0]


# revision 10
# speedup vs baseline: 2.9367x; 2.3925x over previous
"""ArcDecoder Bass kernel for 8 TRN2 NeuronCores.

Math (per node n, with norm_w/norm_b folded into weights host-side):
  zn   = LN(z)
  u1   = relu(zn @ W1eff + b1eff),  u2 = relu(zn @ W2eff + b2eff)
  h1n  = LN(u1), h2n = LN(u2)
  g    = h1n @ Wbil_eff
  score_e = dot(g[a0_e], h2n[a1_e]) + bil_b

Phase A (replicated): every core computes the full g/h2 node tables into its
own DRAM (bf16 matmuls, f32 LN stats).
Phase B (edges sharded E/8): per-edge rows fetched with the dma_gather custom
GPSIMD instruction (1024 rows per instruction, round-robin over 4 SWDGE
queues).  dma_gather takes int16 indices, so node ids >= 32768 gather from a
shifted table base; host groups each core's edges into 4 (head-half,
tail-half) buckets so every 1024-edge block is half-pure.  DVE multiply +
reduce forms the dots; host adds bil_b and inverse-permutes.
"""

import sys

if "/opt/trn_rl_repo" not in sys.path:
    sys.path.insert(0, "/opt/trn_rl_repo")

import numpy as np
import ml_dtypes

import concourse.bass as bass
import concourse.tile as tile
from concourse import bacc, mybir
from concourse.bass_utils import run_bass_kernel_spmd
from concourse.library_config import mlp
from concourse.masks import make_identity

N, D, E = 50000, 128, 500000
NCORES = 8
P = 128
SHARD_T = 49                     # node tiles per core (sharded phase A)
SHARD = SHARD_T * P              # 6272 rows per core
NT = SHARD_T * NCORES            # 392 node tiles total
NPAD = NT * P                    # 50176
TB = 4                           # node subtiles per z-load / table-store batch
EC = E // NCORES                 # 62500 edges per core
HALF = 32768                     # int16 index ceiling for dma_gather

F32 = mybir.dt.float32
BF16 = mybir.dt.bfloat16
AF = mybir.ActivationFunctionType
ALU = mybir.AluOpType

TABLE_DT = BF16                  # dtype of g/h2 tables (gather payload)
MM_DT = BF16                     # dtype of matmul operands in phase A
BLK = 1024                       # edges per gather block
CB = BLK // P                    # row-chunks per partition in a gather tile
NQ = 4                           # SWDGE queues
EPS = 1e-5

_np_tdt = np.float32 if TABLE_DT == F32 else ml_dtypes.bfloat16
_np_mdt = np.float32 if MM_DT == F32 else ml_dtypes.bfloat16


def _build(block_specs):
    """block_specs: list of (head_half, tail_half) per 1024-edge block."""
    nblk = len(block_specs)
    ecpad = nblk * BLK
    S = BLK // 16

    nc = bacc.Bacc("TRN2", target_bir_lowering=False, debug=False,
                   num_devices=NCORES, num_swdge_queues=NQ)

    z_ext = nc.dram_tensor("z", [SHARD, D], F32, kind="ExternalInput").ap()
    wcat_ext = nc.dram_tensor("wcat", [D, 2 * D], MM_DT, kind="ExternalInput").ap()
    bcat_ext = nc.dram_tensor("bcat", [1, 2 * D], MM_DT, kind="ExternalInput").ap()
    wbil_ext = nc.dram_tensor("wbil", [D, D], MM_DT, kind="ExternalInput").ap()
    idx0_ext = nc.dram_tensor("idx0", [nblk, P, S], mybir.dt.int16,
                              kind="ExternalInput").ap()
    idx1_ext = nc.dram_tensor("idx1", [nblk, P, S], mybir.dt.int16,
                              kind="ExternalInput").ap()
    out_ext = nc.dram_tensor("out", [ecpad], F32, kind="ExternalOutput").ap()

    g_shard = nc.dram_tensor("g_shard", [SHARD, D], TABLE_DT).ap()
    h_shard = nc.dram_tensor("h_shard", [SHARD, D], TABLE_DT).ap()
    g_table = nc.dram_tensor("g_table", [NPAD, D], TABLE_DT, addr_space="Shared").ap()
    h_table = nc.dram_tensor("h_table", [NPAD, D], TABLE_DT, addr_space="Shared").ap()

    with tile.TileContext(nc) as tc:
        with (
            tc.tile_pool(name="const", bufs=1) as const_p,
            tc.tile_pool(name="zload", bufs=2) as zload_p,
            tc.tile_pool(name="work", bufs=3) as work_p,
            tc.tile_pool(name="stat", bufs=4) as stat_p,
            tc.tile_pool(name="tabout", bufs=2) as tabout_p,
            tc.tile_pool(name="psum", bufs=2, space="PSUM") as psum_p,
            tc.tile_pool(name="gather", bufs=6) as gather_p,
            tc.tile_pool(name="idx", bufs=6) as idx_p,
            tc.tile_pool(name="score", bufs=4) as score_p,
        ):
            # ---- constants -------------------------------------------------
            nc.gpsimd.load_library(mlp)
            ident = const_p.tile([P, P], MM_DT)
            make_identity(nc, ident[:])
            wcat_sb = const_p.tile([D, 2 * D], MM_DT)
            nc.sync.dma_start(wcat_sb[:], wcat_ext[:])
            bcat_sb = const_p.tile([1, 2 * D], MM_DT)
            nc.sync.dma_start(bcat_sb[:], bcat_ext[:])
            wbil_sb = const_p.tile([D, D], MM_DT)
            nc.sync.dma_start(wbil_sb[:], wbil_ext[:])
            ones_row = const_p.tile([1, P], MM_DT)
            nc.vector.memset(ones_row[:], 1.0)
            epsc = const_p.tile([P, 1], F32)
            nc.vector.memset(epsc[:], EPS)

            # ---- phase A: node tables (sharded; each core its z-shard) ----
            for b0 in range(0, SHARD_T, TB):
                tb = min(TB, SHARD_T - b0)
                n0 = b0 * P
                zbatch = zload_p.tile([P, TB, D], F32, tag="zbatch")
                nc.sync.dma_start(
                    zbatch[:, :tb, :],
                    z_ext[n0:n0 + tb * P, :].rearrange("(j p) d -> p j d", p=P),
                )
                gbuf = tabout_p.tile([P, TB, D], TABLE_DT, tag="gbuf")
                hbuf = tabout_p.tile([P, TB, D], TABLE_DT, tag="hbuf")

                for jj in range(tb):
                    z_t = zbatch[:, jj, :]
                    st = stat_p.tile([P, 6], F32, tag="st")
                    nc.vector.bn_stats(st[:], z_t)
                    mv = stat_p.tile([P, 2], F32, tag="mv")
                    nc.vector.bn_aggr(mv[:], st[:])
                    sd = stat_p.tile([P, 1], F32, tag="sd")
                    nc.scalar.activation(sd[:], mv[:, 1:2], AF.Sqrt, bias=epsc[:])
                    ri = stat_p.tile([P, 1], F32, tag="ri")
                    nc.vector.reciprocal(ri[:], sd[:])
                    zn = work_p.tile([P, D], MM_DT, tag="zn")
                    nc.vector.tensor_scalar(zn[:], z_t, mv[:, 0:1], ri[:],
                                            ALU.subtract, ALU.mult)
                    znT_ps = psum_p.tile([P, P], MM_DT, tag="tpos", space="PSUM")
                    nc.tensor.transpose(znT_ps[:], zn[:], ident[:])
                    znT = work_p.tile([P, P], MM_DT, tag="znT")
                    nc.scalar.copy(znT[:], znT_ps[:])
                    u12_ps = psum_p.tile([P, 2 * D], F32, tag="mm12", space="PSUM")
                    nc.tensor.matmul(u12_ps[:], lhsT=znT[:], rhs=wcat_sb[:],
                                     start=True, stop=False)
                    nc.tensor.matmul(u12_ps[:], lhsT=ones_row[:], rhs=bcat_sb[:],
                                     start=False, stop=True)
                    u12 = work_p.tile([P, 2 * D], F32, tag="u12")
                    nc.scalar.activation(u12[:], u12_ps[:], AF.Relu)
                    st1 = stat_p.tile([P, 6], F32, tag="st1")
                    nc.vector.bn_stats(st1[:], u12[:, 0:D])
                    mv1 = stat_p.tile([P, 2], F32, tag="mv1")
                    nc.vector.bn_aggr(mv1[:], st1[:])
                    sd1 = stat_p.tile([P, 1], F32, tag="sd1")
                    nc.scalar.activation(sd1[:], mv1[:, 1:2], AF.Sqrt, bias=epsc[:])
                    ri1 = stat_p.tile([P, 1], F32, tag="ri1")
                    nc.vector.reciprocal(ri1[:], sd1[:])
                    st2 = stat_p.tile([P, 6], F32, tag="st2")
                    nc.vector.bn_stats(st2[:], u12[:, D:2 * D])
                    mv2 = stat_p.tile([P, 2], F32, tag="mv2")
                    nc.vector.bn_aggr(mv2[:], st2[:])
                    sd2 = stat_p.tile([P, 1], F32, tag="sd2")
                    nc.scalar.activation(sd2[:], mv2[:, 1:2], AF.Sqrt, bias=epsc[:])
                    ri2 = stat_p.tile([P, 1], F32, tag="ri2")
                    nc.vector.reciprocal(ri2[:], sd2[:])
                    h1n = work_p.tile([P, D], MM_DT, tag="h1n")
                    nc.vector.tensor_scalar(h1n[:], u12[:, 0:D], mv1[:, 0:1],
                                            ri1[:], ALU.subtract, ALU.mult)
                    nc.vector.tensor_scalar(hbuf[:, jj, :], u12[:, D:2 * D],
                                            mv2[:, 0:1], ri2[:],
                                            ALU.subtract, ALU.mult)
                    h1nT_ps = psum_p.tile([P, P], MM_DT, tag="tpos", space="PSUM")
                    nc.tensor.transpose(h1nT_ps[:], h1n[:], ident[:])
                    h1nT = work_p.tile([P, P], MM_DT, tag="h1nT")
                    nc.scalar.copy(h1nT[:], h1nT_ps[:])
                    g_ps = psum_p.tile([P, D], F32, tag="gmm", space="PSUM")
                    nc.tensor.matmul(g_ps[:], lhsT=h1nT[:], rhs=wbil_sb[:],
                                     start=True, stop=True)
                    nc.scalar.copy(gbuf[:, jj, :], g_ps[:])

                nc.sync.dma_start(
                    g_shard[n0:n0 + tb * P, :].rearrange("(j p) d -> p j d", p=P),
                    gbuf[:, :tb, :],
                )
                nc.sync.dma_start(
                    h_shard[n0:n0 + tb * P, :].rearrange("(j p) d -> p j d", p=P),
                    hbuf[:, :tb, :],
                )

            # ---- all-gather shards into full tables -----------------------
            nc.gpsimd.collective_compute(
                "AllGather", ALU.bypass,
                replica_groups=[list(range(NCORES))],
                ins=[g_shard[:]], outs=[g_table[:]],
            )
            nc.gpsimd.collective_compute(
                "AllGather", ALU.bypass,
                replica_groups=[list(range(NCORES))],
                ins=[h_shard[:]], outs=[h_table[:]],
            )

            # ---- phase B: dma_gather + dot --------------------------------
            for b, (h0, h1) in enumerate(block_specs):
                i0 = idx_p.tile([P, S], mybir.dt.int16, tag="i0")
                nc.sync.dma_start(i0[:], idx0_ext[b])
                i1 = idx_p.tile([P, S], mybir.dt.int16, tag="i1")
                nc.sync.dma_start(i1[:], idx1_ext[b])
                g_src = g_table[HALF:, :] if h0 else g_table[:, :]
                h_src = h_table[HALF:, :] if h1 else h_table[:, :]
                gt = gather_p.tile([P, CB, D], TABLE_DT, tag="gt")
                nc.gpsimd.dma_gather(gt[:], g_src, i0[:], BLK, BLK, D,
                                     queue_num=(2 * b) % NQ)
                ht = gather_p.tile([P, CB, D], TABLE_DT, tag="ht")
                nc.gpsimd.dma_gather(ht[:], h_src, i1[:], BLK, BLK, D,
                                     queue_num=(2 * b + 1) % NQ)
                nc.vector.tensor_tensor(gt[:], gt[:], ht[:], op=ALU.mult)
                sc = score_p.tile([P, CB], F32, tag="sc")
                nc.vector.tensor_reduce(
                    sc[:], gt[:], axis=mybir.AxisListType.X, op=ALU.add,
                )
                # edge k of block b sits at [k % 128, k // 128]
                nc.sync.dma_start(
                    out_ext[b * BLK:(b + 1) * BLK].rearrange("(j p) -> p j", p=P),
                    sc[:],
                )

    nc.compile()
    return nc


_CACHE = {}
_RUN_KWARGS = {}
LAST_RESULTS = None


def _pack_idx(vals):
    """[nblk, 1024] int16 -> dma_gather SBUF layout [nblk, 128, 64]:
    index k lives at partition k%16, column k//16, replicated into all
    eight 16-partition groups."""
    nblk = vals.shape[0]
    w = vals.reshape(nblk, BLK // 16, 16).transpose(0, 2, 1)   # [nblk,16,S]
    return np.tile(w, (1, 8, 1)).astype(np.int16)


def kernel(**inputs) -> np.ndarray:
    z = np.asarray(inputs["z"], np.float32)
    pot_arcs = np.asarray(inputs["pot_arcs"])
    lin1_w = np.asarray(inputs["lin1_w"], np.float32)
    lin1_b = np.asarray(inputs["lin1_b"], np.float32)
    lin2_w = np.asarray(inputs["lin2_w"], np.float32)
    lin2_b = np.asarray(inputs["lin2_b"], np.float32)
    bil_w = np.asarray(inputs["bil_w"], np.float32)
    bil_b = np.asarray(inputs["bil_b"], np.float32)
    norm_w = np.asarray(inputs["norm_w"], np.float32)
    norm_b = np.asarray(inputs["norm_b"], np.float32)

    if not np.allclose(norm_b, 0.0):
        # general norm_b adds per-node scalar terms; not exercised by this
        # problem's inputs.  Exact numpy fallback keeps kernel() total.
        return _numpy_reference(z, pot_arcs, lin1_w, lin1_b, lin2_w, lin2_b,
                                bil_w, bil_b, norm_w, norm_b)

    w1eff = norm_w[:, None] * lin1_w.T
    b1eff = norm_b @ lin1_w.T + lin1_b
    w2eff = norm_w[:, None] * lin2_w.T
    b2eff = norm_b @ lin2_w.T + lin2_b
    wbil = bil_w[0] * norm_w[None, :]
    wcat = np.concatenate([w1eff, w2eff], axis=1).astype(_np_mdt)
    bcat = np.concatenate([b1eff, b2eff])[None, :].astype(_np_mdt)
    wbil = wbil.astype(_np_mdt)

    zpad = np.zeros((NPAD, D), np.float32)
    zpad[:N] = z

    a0 = pot_arcs[:, 0].astype(np.int32)
    a1 = pot_arcs[:, 1].astype(np.int32)

    # --- bucket each core's edges by (head-half, tail-half) ----------------
    core_data = []
    for c in range(NCORES):
        s = slice(c * EC, (c + 1) * EC)
        a0c, a1c = a0[s], a1[s]
        bucket = (a0c >= HALF) * 2 + (a1c >= HALF)
        order = np.argsort(bucket, kind="stable")
        counts = np.bincount(bucket, minlength=4)
        core_data.append((a0c, a1c, order, counts))

    maxcnt = np.max([cd[3] for cd in core_data], axis=0)
    padded = [int(-(-m // BLK)) * BLK for m in maxcnt]
    offsets = np.concatenate([[0], np.cumsum(padded)])
    ecpad = int(offsets[-1])
    nblk = ecpad // BLK

    block_specs = []
    for bkt in range(4):
        for _ in range(padded[bkt] // BLK):
            block_specs.append((bkt >> 1, bkt & 1))

    in_maps = []
    perms = []
    for c in range(NCORES):
        a0c, a1c, order, counts = core_data[c]
        i0 = np.zeros(ecpad, np.int32)
        i1 = np.zeros(ecpad, np.int32)
        pos = np.zeros(EC, np.int64)
        csum = np.concatenate([[0], np.cumsum(counts)])
        for bkt in range(4):
            sel = order[csum[bkt]:csum[bkt + 1]]
            dst = offsets[bkt] + np.arange(len(sel))
            i0[dst] = a0c[sel] - (HALF if bkt >> 1 else 0)
            i1[dst] = a1c[sel] - (HALF if bkt & 1 else 0)
            pos[sel] = dst
        perms.append(pos)
        in_maps.append({
            "z": zpad[c * SHARD:(c + 1) * SHARD],
            "wcat": wcat,
            "bcat": bcat,
            "wbil": wbil,
            "idx0": _pack_idx(i0.astype(np.int16).reshape(nblk, BLK)),
            "idx1": _pack_idx(i1.astype(np.int16).reshape(nblk, BLK)),
        })

    key = tuple(block_specs)
    if key not in _CACHE:
        _CACHE[key] = _build(block_specs)
    nc = _CACHE[key]

    res = run_bass_kernel_spmd(nc, in_maps, list(range(NCORES)), **_RUN_KWARGS)
    global LAST_RESULTS
    LAST_RESULTS = res

    scores = np.empty(E, np.float32)
    for c in range(NCORES):
        out_c = np.asarray(res.results[c]["out"], np.float32)
        scores[c * EC:(c + 1) * EC] = out_c[perms[c]]
    return scores + float(bil_b[0])


def _numpy_reference(z, pot_arcs, lin1_w, lin1_b, lin2_w, lin2_b,
                     bil_w, bil_b, norm_w, norm_b):
    def ln(x):
        mu = x.mean(-1, keepdims=True)
        var = x.var(-1, keepdims=True)
        return (x - mu) / np.sqrt(var + 1e-5) * norm_w + norm_b

    zn = ln(z)
    h1 = ln(np.maximum(zn @ lin1_w.T + lin1_b, 0.0))
    h2 = ln(np.maximum(zn @ lin2_w.T + lin2_b, 0.0))
    g = h1 @ bil_w[0]
    a0 = pot_arcs[:, 0].astype(np.int64)
    a1 = pot_arcs[:, 1].astype(np.int64)
    return np.einsum("ed,ed->e", g[a0], h2[a1]) + bil_b[0]
